# revision 26
# baseline (speedup 1.0000x reference)
"""Trainium2 Bass kernel: SNN Leaky-Integrate-and-Fire layer.

Computes, for x [T=1024, N_IN=4096] f32 and W [N_OUT=4096, N_IN=4096] f32:
    cur = x @ W.T                                    # [T, N_OUT]
    mem_t = 0.9*mem_{t-1} + cur_t - (mem_{t-1} > 1)  # scan over T
    spk_t = (mem_t > 1)
returning (spk_rec, mem_rec), both [T, N_OUT] f32.

Sharding: N_OUT split across 8 NeuronCores (512 neurons each); x replicated.

Device algorithm per core (neurons on partitions, time along free dim):
  cur[o, t] accumulated in PSUM: 4 o-tiles of 128 partitions x T=1024
  (2 half-banks of 512). Matmul runs a single pass in fp16 (default;
  1 cycle/row on the PE like bf16 but with 10 mantissa bits, enough for
  the +-37-sigma currents vs the unit threshold) or fp32r / bf16-split3.
  The T-scan runs as ONE custom DVE instruction per o-tile (LIF3_ANT:
  3 cycles/element, state held in ALU-stage flops), then spikes are a
  single is_gt and results DMA out.

  Loop shape: first half of T streams x k-tile by k-tile (k-outer,
  o-inner) while the second half's x and all weights load in the
  background; the second half runs o-outer so ps[o] finishes early and
  each o-tile's scan overlaps the next o-tile's matmuls.
"""

import numpy as np

T = 1024
N_IN = 4096
N_OUT = 4096
N_CORES = 8
O_SHARD = N_OUT // N_CORES  # 512
KT = N_IN // 128  # 32 k-tiles
OT = O_SHARD // 128  # 4 o-tiles
BETA = 0.9
THRESHOLD = 1.0

_CACHE = {}

LIF_OP_NAME = "LIF3_ANT"


def _build_lif3_uops():
    """Custom DVE uop program for the LIF recurrence (3 cycles/element):

        m' = beta*m + c[i] - (m > 1)    (beta = CONST_0/s0, init m = CONST_1/s1)
        out[i] = m'

    HW semantics (verified against the stock tensor_tensor_scan program):
    SRC_0 data is only presented to uops with require_inp0 (which also
    consumes), and the scan feedback idiom is block k+1 writing its a-flop,
    read by block k of a later element via NEXT_ALU_OUT_A.

      Pa (consume, write): block0: t = c - r     (c on ALU lane, r = blk1.a)
                           block1: m = t + u     (u = blk2.a)
                           block2: u' = m*beta -> a-flop; delay3 <- m
                           output DELAY_3 (stage 7)
      Pb: block1: r' = IS_GT(CURR_ALU_OUT=m, 1.0) -> a-flop
      Pc: bubble (r' must settle one cycle before the next Pa reads it)
    Seeds: S1 block1 <- m_init, block2 u0 = m_init*beta -> a;
           S2 block1 r0 -> a;  S3 bubble.
    """
    from concourse.dve_uop import (
        ENABLE,
        AluInp,
        AluOp,
        DelayInp,
        InpSel,
        OutPath,
        OutSel,
        Trigger,
        UopConfig,
    )

    def mk(src=False, write=False, trigger=None, nxt=None):
        u = UopConfig()
        if src:
            u.enable_input(InpSel.SRC_0, 0)  # c on the ALU lane
            u.require_inp0 = ENABLE
        u.enable_input(InpSel.CONST_0, 2)  # chain1: beta
        u.enable_input(InpSel.ONE_F32, 3)  # chain2: threshold 1.0
        u.enable_input(InpSel.CONST_1, 5)  # chain4: initial m
        for b_i, b in enumerate(u.datapath_config):
            b.pass_through_delay(1, 2, 4)
            if b_i >= 3:
                b.pass_through_delay(3)
        if write:
            u.enable_output(OutSel.DELAY_3, OutPath.WR0_LO)
        u.trigger = trigger
        u.next_uop = nxt
        u.repeat_count = 1
        return u

    T_ = Trigger

    def cnt(n):
        return ((T_.COUNT, T_.NONE, T_.NONE), (n, 0, 0))

    uops = []
    # 0: S1 — block1 <- m_init; block2: u0 = m_init*beta -> a-flop
    tr, nx = cnt(1)
    u = mk(trigger=tr, nxt=nx)
    u.datapath_config[1].enable_alu(AluOp.BYPASS, AluInp.PREV_DELAY_4)
    u.datapath_config[2].enable_alu(
        AluOp.MULTIPLY, AluInp.PREV_ALU_OUT, AluInp.PREV_DELAY_1
    )
    u.datapath_config[2].alu_out_a_enable = ENABLE
    uops.append(u)
    # 1: S2 — block1: r0 = IS_GT(CURR=m_init, 1.0) -> a-flop
    tr, nx = cnt(2)
    u = mk(trigger=tr, nxt=nx)
    u.datapath_config[1].enable_alu(
        AluOp.IS_GT, AluInp.CURR_ALU_OUT, AluInp.PREV_DELAY_2
    )
    u.datapath_config[1].alu_out_a_enable = ENABLE
    uops.append(u)
    # 2: S3 — bubble
    tr, nx = cnt(3)
    uops.append(mk(trigger=tr, nxt=nx))
    # 3: Pa — consume+write: t=c-r; m=t+u; u'=m*beta; delay3<-m
    u = mk(
        src=True,
        write=True,
        trigger=(T_.SRC_TENSOR_DONE, T_.COUNT, T_.NONE),
        nxt=(0, 4, 0),
    )
    u.datapath_config[0].enable_alu(
        AluOp.SUBTRACT, AluInp.PREV_ALU_OUT, AluInp.NEXT_ALU_OUT_A
    )
    u.datapath_config[1].enable_alu(
        AluOp.ADD, AluInp.PREV_ALU_OUT, AluInp.NEXT_ALU_OUT_A
    )
    u.datapath_config[2].enable_alu(
        AluOp.MULTIPLY, AluInp.PREV_ALU_OUT, AluInp.PREV_DELAY_1
    )
    u.datapath_config[2].alu_out_a_enable = ENABLE
    u.datapath_config[2].enable_delay_from_src(DelayInp.PREV_ALU_OUT, 3)
    uops.append(u)
    # 4: Pb — block1: r' = IS_GT(CURR=m, 1.0) -> a-flop
    tr, nx = cnt(5)
    u = mk(trigger=tr, nxt=nx)
    u.datapath_config[1].enable_alu(
        AluOp.IS_GT, AluInp.CURR_ALU_OUT, AluInp.PREV_DELAY_2
    )
    u.datapath_config[1].alu_out_a_enable = ENABLE
    uops.append(u)
    # 5: Pc — bubble
    tr, nx = cnt(3)
    uops.append(mk(trigger=tr, nxt=nx))
    for u in uops:
        u.validate("v3")
    return uops


def _build_lif3c_uops():
    """LIF3 variant: initial m comes from SRC_1's single element (per
    partition) instead of CONST_1 — used to chain a scan across T-splits.
    Only S1 differs: lane 5 routes SRC_1 and S1 consumes it."""
    from concourse.dve_uop import ENABLE, InpSel

    uops = _build_lif3_uops()
    s1 = uops[0]
    s1.enable_input(InpSel.SRC_1, 5)  # chain4: initial m from src1
    s1.require_inp1 = ENABLE
    return uops


def _register_lif_ops():
    import numpy as np_

    import concourse.dve_ops as dve_ops
    from concourse.dve_ops import DveOp
    from concourse.dve_spec import Spec, Src0
    from concourse.dve_uop import DveOpSpec

    if "lif_ops" in _CACHE:
        return _CACHE["lif_ops"]

    def _ref(in0, in1, c0, c1, c2):
        out = np_.empty_like(in0)
        if in1 is not None:
            m = in1[:, 0].astype(np_.float32)
        else:
            m = np_.full((in0.shape[0],), c1, np_.float32)
        for t in range(in0.shape[1]):
            r = (m > 1.0).astype(np_.float32)
            m = np_.float32(c0) * m + (in0[:, t] - r)
            out[:, t] = m
        return out

    def _make(name, uops_fn, rd1):
        class _RawDveOp(DveOp):
            def compile(self, ver):
                assert ver == "v3", "LIF ops are v3/TRN2-only"
                return DveOpSpec(
                    name=self.name,
                    opcode=dve_ops.get_dve_sub_opcode(self.name),
                    uops=uops_fn(),
                    rd1_en=rd1,
                )

        op = _RawDveOp(
            name=name, spec=Spec(body=Src0, reference=_ref), subdim=False, uops_sha={}
        )
        dve_ops.OPS.append(op)
        dve_ops._SUB_OPCODE_FOR_NAME[op.name] = (
            dve_ops._CUSTOM_DVE_ROW_BASE + len(dve_ops.OPS) - 1
        )
        dve_ops.CUSTOM_DVE_SPECS[op.name] = op.spec
        return op

    ops = (
        _make(LIF_OP_NAME, _build_lif3_uops, False),
        _make("LIF3C_ANT", _build_lif3c_uops, True),
    )
    _CACHE["lif_ops"] = ops
    return ops


def _build_nc(mode: str):
    import concourse.bacc as bacc
    import concourse.mybir as mybir
    from concourse.tile import TileContext

    F32 = mybir.dt.float32
    split3 = mode == "split3"
    MMDT = {
        "fp16": mybir.dt.float16,
        "f32r": mybir.dt.float32r,
        "bf16": mybir.dt.bfloat16,
        "split3": mybir.dt.bfloat16,
    }[mode]
    Op = mybir.AluOpType
    lif_op, lifc_op = _register_lif_ops()
    terms = [(0, 0), (1, 0), (0, 1)] if split3 else [(0, 0)]  # (h_w, h_x)
    NH = 2 if split3 else 1

    nc = bacc.Bacc(target_bir_lowering=False)
    xshape = [2, N_IN, T] if split3 else [N_IN, T]
    wshape = [2, N_IN, O_SHARD] if split3 else [N_IN, O_SHARD]
    F16 = mybir.dt.float16
    U8 = mybir.dt.uint8
    xT_d = nc.dram_tensor("xT", xshape, MMDT, kind="ExternalInput")
    WT_d = nc.dram_tensor("WT", wshape, MMDT, kind="ExternalInput")
    spk_d = nc.dram_tensor("spk", [O_SHARD, T], U8, kind="ExternalOutput")
    mem_d = nc.dram_tensor("mem", [O_SHARD, T], F16, kind="ExternalOutput")

    with TileContext(nc) as tc:
        with (
            tc.tile_pool(name="sb", bufs=1) as sb,
            tc.tile_pool(name="psp", bufs=1, space="PSUM") as psp,
        ):
            # Everything resident in SBUF; all loads enqueued upfront in
            # exact consumption order. Weights go on the Activation HWDGE
            # queue (front-loaded), x on the SP queue (spread over the whole
            # run), so neither queue saturates and the PE never stalls.
            # tiny priming transfers to absorb the DMA-engine cold start
            prime = sb.tile([128, 2, 8], MMDT, name="prime")
            if split3:
                nc.sync.dma_start(prime[:, 0, :], xT_d[0, 0:128, 0:8])
                nc.scalar.dma_start(prime[:, 1, :], xT_d[0, 0:128, 8:16])
            else:
                nc.sync.dma_start(prime[:, 0, :], xT_d[0:128, 0:8])
                nc.scalar.dma_start(prime[:, 1, :], xT_d[0:128, 8:16])

            wt = sb.tile([128, NH, KT, O_SHARD], MMDT, name="wt")
            if split3:
                wt_view = WT_d.rearrange("h (k p) o -> p h k o", p=128)
            else:
                wt_view = WT_d.rearrange("(k p) o -> p () k o", p=128)
            wt_chunks = [(0, 2), (2, 4)] + [(kc, kc + 4) for kc in range(4, KT, 4)]
            for kl, kr in wt_chunks:
                for h in range(NH):
                    nc.scalar.dma_start(wt[:, h, kl:kr, :], wt_view[:, h, kl:kr, :])

            xsb = sb.tile([128, NH, KT, T], MMDT, name="xsb")
            if split3:
                xv = xT_d.rearrange("h (k p) t -> p h k t", p=128)
            else:
                xv = xT_d.rearrange("(k p) t -> p () k t", p=128)
            x0_chunks = [(kk, kk + 2) for kk in range(0, KT, 2)]
            for kl, kr in x0_chunks:
                nc.sync.dma_start(
                    xsb[:, :, kl:kr, 0:512], xv[:, :, kl:kr, 0:512]
                )
            for kc in range(0, KT, 4):
                nc.sync.dma_start(
                    xsb[:, :, kc : kc + 4, 512:T], xv[:, :, kc : kc + 4, 512:T]
                )

            ps = [
                psp.tile([128, T], F32, name=f"ps{o}", tag=f"ps{o}") for o in range(OT)
            ]

            # HAM pre-warm: dependency-free matmuls on scratch SBUF while the
            # first real DMAs are still in flight, so the PE clock gate is
            # already at 8/8 when the real stream starts. Results land in a
            # region that a later start=True group overwrites.
            warm_w = sb.tile([128, 128], MMDT, name="warm_w")
            warm_x = sb.tile([128, 256], MMDT, name="warm_x")
            nc.vector.memset(warm_w, 0.0)
            nc.gpsimd.memset(warm_x, 0.0)
            for i in range(16):
                nc.tensor.matmul(
                    ps[3][:, 512:768],
                    lhsT=warm_w,
                    rhs=warm_x,
                    start=True,
                    stop=True,
                )

            # ---- first half: k-outer ----
            for k in range(KT):
                for o in range(OT):
                    for ti, (hw, hx) in enumerate(terms):
                        nc.tensor.matmul(
                            ps[o][:, 0:512],
                            lhsT=wt[:, hw, k, o * 128 : (o + 1) * 128],
                            rhs=xsb[:, hx, k, 0:512],
                            start=(k == 0 and ti == 0),
                            stop=(k == KT - 1 and ti == len(terms) - 1),
                        )

            # first-half scans + their epilogues run during the second
            # half's matmuls
            M = sb.tile([128, OT, T], F32, name="M")
            M16 = sb.tile([128, OT, T], F16, name="M16")
            Sp = sb.tile([128, OT, T], U8, name="Sp")
            for o in range(OT):
                nc.vector._custom_dve(
                    lif_op, out=M[:, o, 0:512], in0=ps[o][:, 0:512], s0=BETA, s1=0.0
                )
                nc.vector.tensor_scalar(
                    Sp[:, o, 0:512], M[:, o, 0:512], THRESHOLD, None, Op.is_gt
                )
                nc.scalar.copy(M16[:, o, 0:512], M[:, o, 0:512])
                nc.scalar.dma_start(
                    mem_d[o * 128 : (o + 1) * 128, 0:512], M16[:, o, 0:512]
                )
                nc.scalar.dma_start(
                    spk_d[o * 128 : (o + 1) * 128, 0:512], Sp[:, o, 0:512]
                )

            # ---- second half: o-outer so each ps[o] finishes early; the
            # per-o scan + spike + store overlap the next o's matmuls ----
            for o in range(OT):
                for k in range(KT):
                    for ti, (hw, hx) in enumerate(terms):
                        nc.tensor.matmul(
                            ps[o][:, 512:T],
                            lhsT=wt[:, hw, k, o * 128 : (o + 1) * 128],
                            rhs=xsb[:, hx, k, 512:T],
                            start=(k == 0 and ti == 0),
                            stop=(k == KT - 1 and ti == len(terms) - 1),
                        )
                nc.vector._custom_dve(
                    lifc_op,
                    out=M[:, o, 512:T],
                    in0=ps[o][:, 512:T],
                    in1=M[:, o, 511:512],
                    s0=BETA,
                )
                nc.vector.tensor_scalar(
                    Sp[:, o, 512:T], M[:, o, 512:T], THRESHOLD, None, Op.is_gt
                )
                nc.scalar.copy(M16[:, o, 512:T], M[:, o, 512:T])
                nc.scalar.dma_start(
                    mem_d[o * 128 : (o + 1) * 128, 512:T], M16[:, o, 512:T]
                )
                nc.scalar.dma_start(
                    spk_d[o * 128 : (o + 1) * 128, 512:T], Sp[:, o, 512:T]
                )
    nc.finalize()
    return nc


def _get_nc(mode: str):
    if mode not in _CACHE:
        _CACHE[mode] = _build_nc(mode)
    return _CACHE[mode]


def run(x, W, mode="fp16", trace=False):
    import ml_dtypes

    from concourse.bass_utils import run_bass_kernel_spmd

    nc = _get_nc(mode)
    x = np.asarray(x, dtype=np.float32)
    W = np.asarray(W, dtype=np.float32)
    in_maps = []
    if mode == "split3":
        bf16 = ml_dtypes.bfloat16
        x_hi = x.astype(bf16)
        x_lo = (x - x_hi.astype(np.float32)).astype(bf16)
        xT = np.ascontiguousarray(np.stack([x_hi.T, x_lo.T], axis=0))
        W_hi = W.astype(bf16)
        W_lo = (W - W_hi.astype(np.float32)).astype(bf16)
        for c in range(N_CORES):
            sl = slice(c * O_SHARD, (c + 1) * O_SHARD)
            in_maps.append(
                {
                    "xT": xT,
                    "WT": np.ascontiguousarray(np.stack([W_hi[sl].T, W_lo[sl].T], 0)),
                }
            )
    else:
        npdt = {
            "fp16": np.float16,
            "bf16": ml_dtypes.bfloat16,
            "f32r": np.float32,
        }[mode]
        xT = np.ascontiguousarray(x.T.astype(npdt))
        for c in range(N_CORES):
            WTc = np.ascontiguousarray(W[c * O_SHARD : (c + 1) * O_SHARD].T.astype(npdt))
            in_maps.append({"xT": xT, "WT": WTc})
    res = run_bass_kernel_spmd(nc, in_maps, core_ids=list(range(N_CORES)), trace=trace)
    spk = np.concatenate([r["spk"] for r in res.results], axis=0).T.astype(np.float32)
    mem = np.concatenate([r["mem"] for r in res.results], axis=0).T.astype(np.float32)
    return (
        np.ascontiguousarray(spk),
        np.ascontiguousarray(mem),
    ), res


def kernel(x, W):
    out, _ = run(x, W)
    return out


# revision 30
# speedup vs baseline: 1.0295x; 1.0295x over previous
"""Trainium2 Bass kernel: SNN Leaky-Integrate-and-Fire layer.

Computes, for x [T=1024, N_IN=4096] f32 and W [N_OUT=4096, N_IN=4096] f32:
    cur = x @ W.T                                    # [T, N_OUT]
    mem_t = 0.9*mem_{t-1} + cur_t - (mem_{t-1} > 1)  # scan over T
    spk_t = (mem_t > 1)
returning (spk_rec, mem_rec), both [T, N_OUT] f32.

Sharding: N_OUT split across 8 NeuronCores (512 neurons each); x replicated.

Device algorithm per core (neurons on partitions, time along free dim):
  cur[o, t] accumulated in PSUM: 4 o-tiles of 128 partitions x T=1024
  (2 half-banks of 512). Matmul runs a single pass in fp16 (default;
  1 cycle/row on the PE like bf16 but with 10 mantissa bits, enough for
  the +-37-sigma currents vs the unit threshold) or fp32r / bf16-split3.
  The T-scan runs as ONE custom DVE instruction per o-tile (LIF3_ANT:
  3 cycles/element, state held in ALU-stage flops), then spikes are a
  single is_gt and results DMA out.

  Loop shape: first half of T streams x k-tile by k-tile (k-outer,
  o-inner) while the second half's x and all weights load in the
  background; the second half runs o-outer so ps[o] finishes early and
  each o-tile's scan overlaps the next o-tile's matmuls.
"""

import numpy as np

T = 1024
N_IN = 4096
N_OUT = 4096
N_CORES = 8
O_SHARD = N_OUT // N_CORES  # 512
KT = N_IN // 128  # 32 k-tiles
OT = O_SHARD // 128  # 4 o-tiles
BETA = 0.9
THRESHOLD = 1.0

_CACHE = {}

LIF_OP_NAME = "LIF3_ANT"


def _build_lif3_uops():
    """Custom DVE uop program for the LIF recurrence (3 cycles/element):

        m' = beta*m + c[i] - (m > 1)    (beta = CONST_0/s0, init m = CONST_1/s1)
        out[i] = m'

    HW semantics (verified against the stock tensor_tensor_scan program):
    SRC_0 data is only presented to uops with require_inp0 (which also
    consumes), and the scan feedback idiom is block k+1 writing its a-flop,
    read by block k of a later element via NEXT_ALU_OUT_A.

      Pa (consume, write): block0: t = c - r     (c on ALU lane, r = blk1.a)
                           block1: m = t + u     (u = blk2.a)
                           block2: u' = m*beta -> a-flop; delay3 <- m
                           output DELAY_3 (stage 7)
      Pb: block1: r' = IS_GT(CURR_ALU_OUT=m, 1.0) -> a-flop
      Pc: bubble (r' must settle one cycle before the next Pa reads it)
    Seeds: S1 block1 <- m_init, block2 u0 = m_init*beta -> a;
           S2 block1 r0 -> a;  S3 bubble.
    """
    from concourse.dve_uop import (
        ENABLE,
        AluInp,
        AluOp,
        DelayInp,
        InpSel,
        OutPath,
        OutSel,
        Trigger,
        UopConfig,
    )

    def mk(src=False, write=False, trigger=None, nxt=None):
        u = UopConfig()
        if src:
            u.enable_input(InpSel.SRC_0, 0)  # c on the ALU lane
            u.require_inp0 = ENABLE
        u.enable_input(InpSel.CONST_0, 2)  # chain1: beta
        u.enable_input(InpSel.ONE_F32, 3)  # chain2: threshold 1.0
        u.enable_input(InpSel.CONST_1, 5)  # chain4: initial m
        for b_i, b in enumerate(u.datapath_config):
            b.pass_through_delay(1, 2, 4)
            if b_i >= 3:
                b.pass_through_delay(3)
        if write:
            u.enable_output(OutSel.DELAY_3, OutPath.WR0_LO)
        u.trigger = trigger
        u.next_uop = nxt
        u.repeat_count = 1
        return u

    T_ = Trigger

    def cnt(n):
        return ((T_.COUNT, T_.NONE, T_.NONE), (n, 0, 0))

    uops = []
    # 0: S1 — block1 <- m_init; block2: u0 = m_init*beta -> a-flop
    tr, nx = cnt(1)
    u = mk(trigger=tr, nxt=nx)
    u.datapath_config[1].enable_alu(AluOp.BYPASS, AluInp.PREV_DELAY_4)
    u.datapath_config[2].enable_alu(
        AluOp.MULTIPLY, AluInp.PREV_ALU_OUT, AluInp.PREV_DELAY_1
    )
    u.datapath_config[2].alu_out_a_enable = ENABLE
    uops.append(u)
    # 1: S2 — block1: r0 = IS_GT(CURR=m_init, 1.0) -> a-flop
    tr, nx = cnt(2)
    u = mk(trigger=tr, nxt=nx)
    u.datapath_config[1].enable_alu(
        AluOp.IS_GT, AluInp.CURR_ALU_OUT, AluInp.PREV_DELAY_2
    )
    u.datapath_config[1].alu_out_a_enable = ENABLE
    uops.append(u)
    # 2: S3 — bubble
    tr, nx = cnt(3)
    uops.append(mk(trigger=tr, nxt=nx))
    # 3: Pa — consume+write: t=c-r; m=t+u; u'=m*beta; delay3<-m
    u = mk(
        src=True,
        write=True,
        trigger=(T_.SRC_TENSOR_DONE, T_.COUNT, T_.NONE),
        nxt=(0, 4, 0),
    )
    u.datapath_config[0].enable_alu(
        AluOp.SUBTRACT, AluInp.PREV_ALU_OUT, AluInp.NEXT_ALU_OUT_A
    )
    u.datapath_config[1].enable_alu(
        AluOp.ADD, AluInp.PREV_ALU_OUT, AluInp.NEXT_ALU_OUT_A
    )
    u.datapath_config[2].enable_alu(
        AluOp.MULTIPLY, AluInp.PREV_ALU_OUT, AluInp.PREV_DELAY_1
    )
    u.datapath_config[2].alu_out_a_enable = ENABLE
    u.datapath_config[2].enable_delay_from_src(DelayInp.PREV_ALU_OUT, 3)
    uops.append(u)
    # 4: Pb — block1: r' = IS_GT(CURR=m, 1.0) -> a-flop
    tr, nx = cnt(5)
    u = mk(trigger=tr, nxt=nx)
    u.datapath_config[1].enable_alu(
        AluOp.IS_GT, AluInp.CURR_ALU_OUT, AluInp.PREV_DELAY_2
    )
    u.datapath_config[1].alu_out_a_enable = ENABLE
    uops.append(u)
    # 5: Pc — bubble
    tr, nx = cnt(3)
    uops.append(mk(trigger=tr, nxt=nx))
    for u in uops:
        u.validate("v3")
    return uops


def _build_lif3c_uops():
    """LIF3 variant: initial m comes from SRC_1's single element (per
    partition) instead of CONST_1 — used to chain a scan across T-splits.
    Only S1 differs: lane 5 routes SRC_1 and S1 consumes it."""
    from concourse.dve_uop import ENABLE, InpSel

    uops = _build_lif3_uops()
    s1 = uops[0]
    s1.enable_input(InpSel.SRC_1, 5)  # chain4: initial m from src1
    s1.require_inp1 = ENABLE
    return uops


def _register_lif_ops():
    import numpy as np_

    import concourse.dve_ops as dve_ops
    from concourse.dve_ops import DveOp
    from concourse.dve_spec import Spec, Src0
    from concourse.dve_uop import DveOpSpec

    if "lif_ops" in _CACHE:
        return _CACHE["lif_ops"]

    def _ref(in0, in1, c0, c1, c2):
        out = np_.empty_like(in0)
        if in1 is not None:
            m = in1[:, 0].astype(np_.float32)
        else:
            m = np_.full((in0.shape[0],), c1, np_.float32)
        for t in range(in0.shape[1]):
            r = (m > 1.0).astype(np_.float32)
            m = np_.float32(c0) * m + (in0[:, t] - r)
            out[:, t] = m
        return out

    def _make(name, uops_fn, rd1):
        class _RawDveOp(DveOp):
            def compile(self, ver):
                assert ver == "v3", "LIF ops are v3/TRN2-only"
                return DveOpSpec(
                    name=self.name,
                    opcode=dve_ops.get_dve_sub_opcode(self.name),
                    uops=uops_fn(),
                    rd1_en=rd1,
                )

        op = _RawDveOp(
            name=name, spec=Spec(body=Src0, reference=_ref), subdim=False, uops_sha={}
        )
        dve_ops.OPS.append(op)
        dve_ops._SUB_OPCODE_FOR_NAME[op.name] = (
            dve_ops._CUSTOM_DVE_ROW_BASE + len(dve_ops.OPS) - 1
        )
        dve_ops.CUSTOM_DVE_SPECS[op.name] = op.spec
        return op

    ops = (
        _make(LIF_OP_NAME, _build_lif3_uops, False),
        _make("LIF3C_ANT", _build_lif3c_uops, True),
    )
    _CACHE["lif_ops"] = ops
    return ops


def _build_nc(mode: str):
    import concourse.bacc as bacc
    import concourse.mybir as mybir
    from concourse.tile import TileContext

    F32 = mybir.dt.float32
    split3 = mode == "split3"
    MMDT = {
        "fp16": mybir.dt.float16,
        "f32r": mybir.dt.float32r,
        "bf16": mybir.dt.bfloat16,
        "split3": mybir.dt.bfloat16,
    }[mode]
    Op = mybir.AluOpType
    lif_op, lifc_op = _register_lif_ops()
    terms = [(0, 0), (1, 0), (0, 1)] if split3 else [(0, 0)]  # (h_w, h_x)
    NH = 2 if split3 else 1

    nc = bacc.Bacc(target_bir_lowering=False)
    xshape = [2, N_IN, T] if split3 else [N_IN, T]
    wshape = [2, N_IN, O_SHARD] if split3 else [N_IN, O_SHARD]
    F16 = mybir.dt.float16
    U8 = mybir.dt.uint8
    xT_d = nc.dram_tensor("xT", xshape, MMDT, kind="ExternalInput")
    WT_d = nc.dram_tensor("WT", wshape, MMDT, kind="ExternalInput")
    spk_d = nc.dram_tensor("spk", [O_SHARD, T], U8, kind="ExternalOutput")
    mem_d = nc.dram_tensor("mem", [O_SHARD, T], F16, kind="ExternalOutput")

    with TileContext(nc) as tc:
        with (
            tc.tile_pool(name="sb", bufs=1) as sb,
            tc.tile_pool(name="psp", bufs=1, space="PSUM") as psp,
        ):
            # Everything resident in SBUF; all loads enqueued upfront in
            # exact consumption order. Weights go on the Activation HWDGE
            # queue (front-loaded), x on the SP queue (spread over the whole
            # run), so neither queue saturates and the PE never stalls.
            # tiny priming transfers to absorb the DMA-engine cold start
            prime = sb.tile([128, 2, 8], MMDT, name="prime")
            if split3:
                nc.sync.dma_start(prime[:, 0, :], xT_d[0, 0:128, 0:8])
                nc.scalar.dma_start(prime[:, 1, :], xT_d[0, 0:128, 8:16])
            else:
                nc.sync.dma_start(prime[:, 0, :], xT_d[0:128, 0:8])
                nc.scalar.dma_start(prime[:, 1, :], xT_d[0:128, 8:16])

            wt = sb.tile([128, NH, KT, O_SHARD], MMDT, name="wt")
            if split3:
                wt_view = WT_d.rearrange("h (k p) o -> p h k o", p=128)
            else:
                wt_view = WT_d.rearrange("(k p) o -> p () k o", p=128)
            wt_chunks = [(0, 1), (1, 2), (2, 4)] + [
                (kc, kc + 4) for kc in range(4, KT, 4)
            ]
            for kl, kr in wt_chunks:
                for h in range(NH):
                    nc.scalar.dma_start(wt[:, h, kl:kr, :], wt_view[:, h, kl:kr, :])

            xsb = sb.tile([128, NH, KT, T], MMDT, name="xsb")
            if split3:
                xv = xT_d.rearrange("h (k p) t -> p h k t", p=128)
            else:
                xv = xT_d.rearrange("(k p) t -> p () k t", p=128)
            x0_chunks = [(0, 1), (1, 2)] + [(kk, kk + 2) for kk in range(2, KT, 2)]
            for kl, kr in x0_chunks:
                nc.sync.dma_start(
                    xsb[:, :, kl:kr, 0:512], xv[:, :, kl:kr, 0:512]
                )
            for kc in range(0, KT, 4):
                nc.sync.dma_start(
                    xsb[:, :, kc : kc + 4, 512:T], xv[:, :, kc : kc + 4, 512:T]
                )

            ps = [
                psp.tile([128, T], F32, name=f"ps{o}", tag=f"ps{o}") for o in range(OT)
            ]

            # HAM pre-warm: dependency-free matmuls on scratch SBUF while the
            # first real DMAs are still in flight, so the PE clock gate is
            # already at 8/8 when the real stream starts. Results land in a
            # region that a later start=True group overwrites.
            warm_w = sb.tile([128, 128], MMDT, name="warm_w")
            warm_x = sb.tile([128, 256], MMDT, name="warm_x")
            nc.vector.memset(warm_w, 0.0)
            nc.gpsimd.memset(warm_x, 0.0)
            for i in range(16):
                nc.tensor.matmul(
                    ps[3][:, 512:768],
                    lhsT=warm_w,
                    rhs=warm_x,
                    start=True,
                    stop=True,
                )

            # ---- first half: k-outer ----
            for k in range(KT):
                for o in range(OT):
                    for ti, (hw, hx) in enumerate(terms):
                        nc.tensor.matmul(
                            ps[o][:, 0:512],
                            lhsT=wt[:, hw, k, o * 128 : (o + 1) * 128],
                            rhs=xsb[:, hx, k, 0:512],
                            start=(k == 0 and ti == 0),
                            stop=(k == KT - 1 and ti == len(terms) - 1),
                        )

            # first-half scans + their epilogues run during the second
            # half's matmuls
            M = sb.tile([128, OT, T], F32, name="M")
            M16 = sb.tile([128, OT, T], F16, name="M16")
            Sp = sb.tile([128, OT, T], U8, name="Sp")
            for o in range(OT):
                nc.vector._custom_dve(
                    lif_op, out=M[:, o, 0:512], in0=ps[o][:, 0:512], s0=BETA, s1=0.0
                )
                nc.vector.tensor_scalar(
                    Sp[:, o, 0:512], M[:, o, 0:512], THRESHOLD, None, Op.is_gt
                )
                nc.scalar.copy(M16[:, o, 0:512], M[:, o, 0:512])
                nc.sync.dma_start(
                    mem_d[o * 128 : (o + 1) * 128, 0:512], M16[:, o, 0:512]
                )
                nc.scalar.dma_start(
                    spk_d[o * 128 : (o + 1) * 128, 0:512], Sp[:, o, 0:512]
                )

            # ---- second half: o-outer so each ps[o] finishes early; the
            # per-o scan + spike + store overlap the next o's matmuls ----
            for o in range(OT):
                for k in range(KT):
                    for ti, (hw, hx) in enumerate(terms):
                        nc.tensor.matmul(
                            ps[o][:, 512:T],
                            lhsT=wt[:, hw, k, o * 128 : (o + 1) * 128],
                            rhs=xsb[:, hx, k, 512:T],
                            start=(k == 0 and ti == 0),
                            stop=(k == KT - 1 and ti == len(terms) - 1),
                        )
                nc.vector._custom_dve(
                    lifc_op,
                    out=M[:, o, 512:T],
                    in0=ps[o][:, 512:T],
                    in1=M[:, o, 511:512],
                    s0=BETA,
                )
                nc.vector.tensor_scalar(
                    Sp[:, o, 512:T], M[:, o, 512:T], THRESHOLD, None, Op.is_gt
                )
                nc.scalar.copy(M16[:, o, 512:T], M[:, o, 512:T])
                nc.sync.dma_start(
                    mem_d[o * 128 : (o + 1) * 128, 512:T], M16[:, o, 512:T]
                )
                nc.scalar.dma_start(
                    spk_d[o * 128 : (o + 1) * 128, 512:T], Sp[:, o, 512:T]
                )
    nc.finalize()
    return nc


def _get_nc(mode: str):
    if mode not in _CACHE:
        _CACHE[mode] = _build_nc(mode)
    return _CACHE[mode]


def run(x, W, mode="fp16", trace=False):
    import ml_dtypes

    from concourse.bass_utils import run_bass_kernel_spmd

    nc = _get_nc(mode)
    x = np.asarray(x, dtype=np.float32)
    W = np.asarray(W, dtype=np.float32)
    in_maps = []
    if mode == "split3":
        bf16 = ml_dtypes.bfloat16
        x_hi = x.astype(bf16)
        x_lo = (x - x_hi.astype(np.float32)).astype(bf16)
        xT = np.ascontiguousarray(np.stack([x_hi.T, x_lo.T], axis=0))
        W_hi = W.astype(bf16)
        W_lo = (W - W_hi.astype(np.float32)).astype(bf16)
        for c in range(N_CORES):
            sl = slice(c * O_SHARD, (c + 1) * O_SHARD)
            in_maps.append(
                {
                    "xT": xT,
                    "WT": np.ascontiguousarray(np.stack([W_hi[sl].T, W_lo[sl].T], 0)),
                }
            )
    else:
        npdt = {
            "fp16": np.float16,
            "bf16": ml_dtypes.bfloat16,
            "f32r": np.float32,
        }[mode]
        xT = np.ascontiguousarray(x.T.astype(npdt))
        for c in range(N_CORES):
            WTc = np.ascontiguousarray(W[c * O_SHARD : (c + 1) * O_SHARD].T.astype(npdt))
            in_maps.append({"xT": xT, "WT": WTc})
    res = run_bass_kernel_spmd(nc, in_maps, core_ids=list(range(N_CORES)), trace=trace)
    spk = np.concatenate([r["spk"] for r in res.results], axis=0).T.astype(np.float32)
    mem = np.concatenate([r["mem"] for r in res.results], axis=0).T.astype(np.float32)
    return (
        np.ascontiguousarray(spk),
        np.ascontiguousarray(mem),
    ), res


def kernel(x, W):
    out, _ = run(x, W)
    return out
